# revision 69
# baseline (speedup 1.0000x reference)
"""Mixtral decoder layer on 8 TRN2 NeuronCores.

Sharding:
  - Attention: sequence-parallel. Core c owns tokens [c*128, (c+1)*128).
    Each core computes rmsnorm1 + q/k/v projections + RoPE for its own
    128 tokens, AllGathers the RoPE'd K and V (small), then computes
    causal attention + o-projection + residual for its token block.
  - MoE: expert-parallel, dense-equivalent. Core c owns expert c. After
    rmsnorm2 + router (top-2 weights per token), the normed activations
    are AllGathered in transposed layout [H, T]. Core c computes
    w_te[:, c] * down_c(silu(up_c(x)) * gate_c(x)) for all 1024 tokens;
    a ReduceScatter(add) returns each core its token block of the sum.
  - Heavy matmuls run in float32r (4x fp32 throughput, ~1.5e-4 rel err).
  - ln1_w / ln2_w are folded into downstream weight matrices on host.

Self-contained: hardcodes all shapes from the problem spec.
"""
import os

import ml_dtypes
import numpy as np

import concourse.bass as bass  # noqa: F401
import concourse.mybir as mybir
from concourse import bacc, tile
from concourse.bass_utils import run_bass_kernel_spmd

F32 = mybir.dt.float32
F32R = mybir.dt.float32r
BF16 = mybir.dt.bfloat16
AF = mybir.ActivationFunctionType
ALU = mybir.AluOpType
AX = mybir.AxisListType

NCORES = 8
B, S, H = 1, 1024, 2048
NH, KVH, HD = 16, 4, 128
E, TOPK, F = 8, 2, 4096
EPS = 1e-6
TB = S // NCORES          # tokens per core = 128
HC = H // 128             # 16 contraction chunks over H
FT = F // 128             # 32 F tiles
NEG = -1.0e30
CAP = 384                 # expert token capacity (mean load 256, ~9 sigma)
CB = CAP // 128           # capacity chunks of 128
DH = 256                  # down-proj h-slice per weight load


def build_nc():
    nc = bacc.Bacc(num_devices=NCORES)

    # ---- per-core external inputs ----
    h_in = nc.dram_tensor("h", [TB, H], F32, kind="ExternalInput")
    cos_q = nc.dram_tensor("cos_q", [TB, H], F32, kind="ExternalInput")
    sin_q = nc.dram_tensor("sin_q", [TB, H], F32, kind="ExternalInput")
    cos_k = nc.dram_tensor("cos_k", [TB, KVH * HD], F32, kind="ExternalInput")
    sin_k = nc.dram_tensor("sin_k", [TB, KVH * HD], F32, kind="ExternalInput")
    bias_all = nc.dram_tensor("bias_all", [NCORES, TB, TB], F32, kind="ExternalInput")
    ident_in = nc.dram_tensor("ident", [128, 128], F32, kind="ExternalInput")
    identb_in = nc.dram_tensor("identb", [128, 128], BF16, kind="ExternalInput")
    sel_in = nc.dram_tensor("sel", [E, 128], BF16, kind="ExternalInput")
    esel_in = nc.dram_tensor("esel", [128, E], BF16, kind="ExternalInput")
    ltri_in = nc.dram_tensor("ltri", [128, 128], BF16, kind="ExternalInput")
    onesc_in = nc.dram_tensor("onesc", [128, 1], BF16, kind="ExternalInput")
    ones1f_in = nc.dram_tensor("ones1f", [1, 128], F32, kind="ExternalInput")
    iotac_in = nc.dram_tensor("iotac", [128, CAP], F32, kind="ExternalInput")
    qw = nc.dram_tensor("qw", [4, 128, HC, 512], BF16, kind="ExternalInput")
    kw = nc.dram_tensor("kw", [1, 128, HC, 512], BF16, kind="ExternalInput")
    vw = nc.dram_tensor("vw", [1, 128, HC, 512], BF16, kind="ExternalInput")
    ow = nc.dram_tensor("ow", [4, 128, HC, 512], BF16, kind="ExternalInput")
    rw_in = nc.dram_tensor("rw", [H, E], F32, kind="ExternalInput")
    # expert weights, host-retiled:
    #   upw/gatew: [FT, 128(p=H row in chunk), HC, 128(f)]
    #   downw:     [FT, 128(p=F row in tile), H]
    upw = nc.dram_tensor("upw", [FT, 128, HC, 128], BF16, kind="ExternalInput")
    gatew = nc.dram_tensor("gatew", [FT, 128, HC, 128], BF16, kind="ExternalInput")
    downw = nc.dram_tensor("downw", [FT, 128, H], BF16, kind="ExternalInput")

    out_ext = nc.dram_tensor("out", [TB, H], F32, kind="ExternalOutput")

    # ---- internal DRAM (collective bounce buffers) ----
    ag_kv_in = nc.dram_tensor("ag_kv_in", [128, 1024], BF16)
    ag_kv_out = nc.dram_tensor("ag_kv_out", [NCORES, 128, 1024], BF16,
                               addr_space="Shared")
    ag_xa_in = nc.dram_tensor("ag_xa_in", [TB, E + 1024], BF16)
    ag_xa_out = nc.dram_tensor("ag_xa_out", [NCORES, TB, E + 1024], BF16,
                               addr_space="Shared")
    ag_xb_in = nc.dram_tensor("ag_xb_in", [TB, 1024], BF16)
    ag_xb_out = nc.dram_tensor("ag_xb_out", [NCORES, TB, 1024], BF16,
                               addr_space="Shared")
    y_part = [nc.dram_tensor(f"y_part{i}", [NCORES, TB, 512], BF16) for i in range(4)]
    y_rs = [nc.dram_tensor(f"y_rs{i}", [TB, 512], BF16) for i in range(4)]

    rg = [list(range(NCORES))]

    with tile.TileContext(nc) as tc:
        with (
            tc.tile_pool(name="glob", bufs=1) as glob,
            tc.tile_pool(name="psA", bufs=2, space="PSUM") as psA,
            tc.tile_pool(name="psC", bufs=2, space="PSUM") as psC,
            tc.tile_pool(name="psD", bufs=2, space="PSUM") as psD,
        ):
            h_sb = glob.tile([TB, H], F32, tag="h_sb")
            nc.sync.dma_start(out=h_sb[:], in_=h_in[:, :])
            ident = glob.tile([128, 128], F32, tag="ident")
            nc.sync.dma_start(out=ident[:], in_=ident_in[:, :])
            identb = glob.tile([128, 128], BF16, tag="identb")
            nc.sync.dma_start(out=identb[:], in_=identb_in[:, :])
            x2 = glob.tile([TB, H], F32, tag="x2")
            epsc = glob.tile([TB, 1], F32, tag="epsc")
            nc.vector.memset(epsc[:], EPS)
            # MoE dispatch constants, preloaded so nothing queues behind
            # the big activation DMAs later
            sel_sb = glob.tile([E, 128], BF16, tag="sel_sb")
            nc.sync.dma_start(out=sel_sb[:], in_=sel_in[:, :])
            esel_sb = glob.tile([128, E], BF16, tag="esel_sb")
            nc.sync.dma_start(out=esel_sb[:], in_=esel_in[:, :])
            ltri_sb = glob.tile([128, 128], BF16, tag="ltri_sb")
            nc.sync.dma_start(out=ltri_sb[:], in_=ltri_in[:, :])
            onesc_sb = glob.tile([128, 1], BF16, tag="onesc_sb")
            nc.sync.dma_start(out=onesc_sb[:], in_=onesc_in[:, :])
            ones1f_sb = glob.tile([1, 128], F32, tag="ones1f_sb")
            nc.sync.dma_start(out=ones1f_sb[:], in_=ones1f_in[:, :])
            iotac_sb = glob.tile([128, CAP], F32, tag="iotac_sb")
            nc.sync.dma_start(out=iotac_sb[:], in_=iotac_in[:, :])

            # =============== attention ===============
            with tc.tile_pool(name="at_keep", bufs=1) as akp:
                qr = akp.tile([TB, NH, HD], F32, tag="qr")
                kv_loc = akp.tile([TB, 1024], F32, tag="kv_loc")  # [k | v]

                with (
                    tc.tile_pool(name="at_pre", bufs=1) as pp1,
                    tc.tile_pool(name="at_pre2", bufs=3) as pp2,
                ):
                    # --- rmsnorm1 stats (scale applied at proj copy-out) ---
                    sq = pp1.tile([TB, H], F32, tag="sq")
                    nc.vector.tensor_mul(sq[:], h_sb[:], h_sb[:])
                    var = pp1.tile([TB, 1], F32, tag="var")
                    nc.vector.tensor_reduce(var[:], sq[:], axis=AX.X, op=ALU.add)
                    sd = pp1.tile([TB, 1], F32, tag="sd")
                    nc.scalar.activation(sd[:], var[:], AF.Sqrt, bias=epsc[:], scale=1.0 / H)
                    rs1 = pp1.tile([TB, 1], F32, tag="rs1")
                    nc.vector.reciprocal(rs1[:], sd[:])

                    # --- hT (16 PE transposes of the raw residual) ---
                    x1t = pp1.tile([128, HC, TB], BF16, tag="x1t")
                    for kc in range(HC):
                        pt = psC.tile([128, 128], F32, tag="mid")
                        nc.tensor.transpose(pt[:], h_sb[:, kc * 128:(kc + 1) * 128], ident[:])
                        nc.scalar.copy(x1t[:, kc, :], pt[:])

                    # --- q/k/v projections (out = [tok, dim], scaled by 1/rms) ---
                    q_sb = pp1.tile([TB, NH * HD], F32, tag="q_sb")

                    def proj(w_dram, n_dim, out_fn):
                        for n0 in range(0, n_dim, 512):
                            pp = psC.tile([128, 512], F32, tag="mid")
                            wt = pp2.tile([128, HC, 512], BF16, tag="w_sb")
                            nc.scalar.dma_start(
                                out=wt[:],
                                in_=w_dram[n0 // 512, :, :, :],
                            )
                            for kc in range(HC):
                                nc.tensor.matmul(
                                    pp[:], x1t[:, kc, :], wt[:, kc, :],
                                    start=(kc == 0), stop=(kc == HC - 1),
                                )
                            out_fn(n0, pp[:])

                    def rope(src3, cos3, sin3, dst3, nh):
                        hh = HD // 2
                        a = pp2.tile([TB, NH, hh], F32, tag="rope_t")
                        b2 = pp2.tile([TB, NH, hh], F32, tag="rope_t")
                        nc.vector.tensor_mul(a[:, 0:nh, :], src3[:, :, 0:hh], cos3[:, :, 0:hh])
                        nc.vector.tensor_mul(b2[:, 0:nh, :], src3[:, :, hh:], sin3[:, :, 0:hh])
                        nc.vector.tensor_sub(dst3[:, :, 0:hh], a[:, 0:nh, :], b2[:, 0:nh, :])
                        c2 = pp2.tile([TB, NH, hh], F32, tag="rope_t")
                        d2 = pp2.tile([TB, NH, hh], F32, tag="rope_t")
                        nc.vector.tensor_mul(c2[:, 0:nh, :], src3[:, :, hh:], cos3[:, :, hh:])
                        nc.vector.tensor_mul(d2[:, 0:nh, :], src3[:, :, 0:hh], sin3[:, :, hh:])
                        nc.vector.tensor_add(dst3[:, :, hh:], c2[:, 0:nh, :], d2[:, 0:nh, :])

                    # v then k, so the combined AllGather launches before q
                    proj(vw, KVH * HD,
                         lambda n0, pp: nc.vector.tensor_scalar_mul(
                             kv_loc[:, 512:1024], pp, rs1[:]))
                    kvb = pp1.tile([128, 1024], BF16, tag="kvb")
                    nc.vector.tensor_copy(kvb[:, 512:1024], kv_loc[:, 512:1024])

                    proj(kw, KVH * HD,
                         lambda n0, pp: nc.vector.tensor_scalar_mul(
                             kv_loc[:, 0:512], pp, rs1[:]))
                    ck = pp1.tile([TB, KVH, HD], F32, tag="ck")
                    skv = pp1.tile([TB, KVH, HD], F32, tag="skv")
                    nc.sync.dma_start(out=ck[:], in_=cos_k[:, :].rearrange("t (h d) -> t h d", d=HD))
                    nc.sync.dma_start(out=skv[:], in_=sin_k[:, :].rearrange("t (h d) -> t h d", d=HD))
                    kr = pp1.tile([TB, KVH, HD], F32, tag="kr")
                    rope(kv_loc[:, 0:512].rearrange("t (h d) -> t h d", d=HD), ck, skv, kr[:], KVH)

                    # transpose local K now so receivers skip it: payload
                    # cols g*128:(g+1)*128 hold kT[hd, tok] for group g
                    for g in range(KVH):
                        ptk = psC.tile([128, 512], F32, tag="mid")
                        nc.tensor.transpose(ptk[:, 0:128], kr[:, g, :], ident[:])
                        nc.vector.tensor_copy(kvb[:, g * 128:(g + 1) * 128],
                                              ptk[:, 0:128])
                    nc.sync.dma_start(out=ag_kv_in[:, :], in_=kvb[:])
                    nc.gpsimd.collective_compute(
                        "AllGather", ALU.bypass, replica_groups=rg,
                        ins=[ag_kv_in[:, :].opt()], outs=[ag_kv_out[:, :, :].opt()],
                    )

                    # --- q-proj + RoPE overlap the kv AllGather ---
                    proj(qw, NH * HD,
                         lambda n0, pp: nc.vector.tensor_scalar_mul(
                             q_sb[:, n0:n0 + 512], pp, rs1[:]))
                    cq = pp1.tile([TB, NH, HD], F32, tag="cq")
                    sqv = pp1.tile([TB, NH, HD], F32, tag="sqv")
                    nc.sync.dma_start(out=cq[:], in_=cos_q[:, :].rearrange("t (h d) -> t h d", d=HD))
                    nc.sync.dma_start(out=sqv[:], in_=sin_q[:, :].rearrange("t (h d) -> t h d", d=HD))
                    rope(q_sb[:].rearrange("t (h d) -> t h d", d=HD), cq, sqv, qr[:], NH)

                # --- attention proper ---
                with (
                    tc.tile_pool(name="at_core", bufs=1) as acp,
                    tc.tile_pool(name="at_core2", bufs=2) as acp2,
                ):
                    qt = acp.tile([128, NH, TB], BF16, tag="qt")
                    for hh in range(NH):
                        pt = psC.tile([128, 128], F32, tag="mid")
                        nc.tensor.transpose(pt[:], qr[:, hh, :], ident[:])
                        nc.scalar.copy(qt[:, hh, :], pt[:])

                    bias_sb = acp.tile([TB, NCORES, TB], F32, tag="bias_sb")
                    nc.sync.dma_start(out=bias_sb[:],
                                      in_=bias_all[:, :, :].rearrange("b q k -> q b k"))
                    kt = acp.tile([128, KVH, S], BF16, tag="kt")  # [hd, g, keys]
                    for g in range(KVH):
                        nc.sync.dma_start(
                            out=kt[:, g, :].rearrange("p (b t) -> p b t", t=TB),
                            in_=ag_kv_out[:, :, g * 128:(g + 1) * 128]
                                .rearrange("b p t -> p b t"))
                    v_sb = acp.tile([TB, NCORES, 512], BF16, tag="v_sb")
                    for b in range(NCORES):
                        nc.sync.dma_start(out=v_sb[:, b, :],
                                          in_=ag_kv_out[b, :, 512:1024])

                    attn_ot = acp.tile([128, NH, TB], BF16, tag="attn_ot")  # [hd, head, tok]
                    for hh in range(NH):
                        g = hh // (NH // KVH)
                        ps = psA.tile([TB, S], F32, tag="big")
                        for n0 in range(0, S, 512):
                            nc.tensor.matmul(ps[:, n0:n0 + 512], qt[:, hh, :],
                                             kt[:, g, n0:n0 + 512], start=True,
                                             stop=True)
                        sc_sb = acp2.tile([TB, NCORES, TB], F32, tag="sc_sb")
                        nc.vector.tensor_add(sc_sb[:],
                                             ps[:].rearrange("q (b k) -> q b k", k=TB),
                                             bias_sb[:])
                        flat = sc_sb[:].rearrange("q b k -> q (b k)")
                        esum = acp2.tile([TB, 1], F32, tag="esum")
                        nc.scalar.activation(flat, flat, AF.Exp, bias=0.0, scale=1.0,
                                             accum_out=esum[:])
                        rinv = acp2.tile([TB, 1], F32, tag="rinv")
                        nc.vector.reciprocal(rinv[:], esum[:])
                        prb = acp2.tile([TB, NCORES, TB], BF16, tag="prb")
                        nc.vector.tensor_scalar_mul(
                            prb[:].rearrange("q b k -> q (b k)"), flat, rinv[:])

                        pavt = psC.tile([128, 512], F32, tag="mid")
                        pav = pavt[:, 0:TB]
                        for b in range(NCORES):
                            pt = psD.tile([128, 128], BF16, tag="midb")
                            nc.tensor.transpose(pt[:], prb[:, b, :], identb[:])
                            at_sb = acp2.tile([TB, TB], BF16, tag="at_sb")
                            nc.vector.tensor_copy(at_sb[:], pt[:])
                            nc.tensor.matmul(pav[:],
                                             v_sb[:, b, g * 128:(g + 1) * 128],
                                             at_sb[:], start=(b == 0),
                                             stop=(b == NCORES - 1))
                        nc.scalar.copy(attn_ot[:, hh, :], pav)

                    # --- o projection + residual ---
                    for n0 in range(0, H, 512):
                        po = psC.tile([128, 512], F32, tag="mid")
                        wt = acp2.tile([128, HC, 512], BF16, tag="w_sb2")
                        nc.scalar.dma_start(
                            out=wt[:],
                            in_=ow[n0 // 512, :, :, :])
                        for kc in range(HC):
                            nc.tensor.matmul(po[:], attn_ot[:, kc, :], wt[:, kc, :],
                                             start=(kc == 0), stop=(kc == HC - 1))
                        nc.vector.tensor_add(x2[:, n0:n0 + 512], h_sb[:, n0:n0 + 512], po[:])

            # =============== rmsnorm2 + router + AG ===============
            with tc.tile_pool(name="mid", bufs=1) as mp:
                # transposes of the raw x2 start immediately on the PE;
                # the rmsnorm stats run concurrently on vector/scalar
                xmt = mp.tile([128, HC, TB], F32R, tag="xmt")
                for kc in range(HC):
                    pt = psC.tile([128, 128], F32, tag="mid")
                    nc.tensor.transpose(pt[:], x2[:, kc * 128:(kc + 1) * 128], ident[:])
                    nc.scalar.copy(xmt[:, kc, :], pt[:])

                sq2 = mp.tile([TB, H], F32, tag="sq2")
                nc.vector.tensor_mul(sq2[:], x2[:], x2[:])
                var2 = mp.tile([TB, 1], F32, tag="var2")
                nc.vector.tensor_reduce(var2[:], sq2[:], axis=AX.X, op=ALU.add)
                sd2 = mp.tile([TB, 1], F32, tag="sd2")
                nc.scalar.activation(sd2[:], var2[:], AF.Sqrt, bias=epsc[:], scale=1.0 / H)
                rs2 = mp.tile([TB, 1], F32, tag="rs2")
                nc.vector.reciprocal(rs2[:], sd2[:])
                xm = mp.tile([TB, H], F32, tag="xm")
                nc.vector.tensor_scalar_mul(xm[:], x2[:], rs2[:])

                # router (ln2 folded into rw on host); logits scaled by 1/rms
                rwt = mp.tile([128, HC, E], F32R, tag="rwt")
                nc.sync.dma_start(out=rwt[:],
                                  in_=rw_in[:, :].rearrange("(k p) e -> p k e",
                                                            p=128).bitcast(F32R))
                plt = psC.tile([128, 512], F32, tag="mid")
                pl = plt[0:TB, 0:E]
                for kc in range(HC):
                    nc.tensor.matmul(pl[:], xmt[:, kc, :], rwt[:, kc, :],
                                     start=(kc == 0), stop=(kc == HC - 1))
                lgin = mp.tile([TB, E], F32, tag="lgin")
                nc.vector.tensor_scalar_mul(lgin[:], pl[:], rs2[:])
                lg = mp.tile([TB, E], F32, tag="lg")
                esum2 = mp.tile([TB, 1], F32, tag="esum2")
                nc.scalar.activation(lg[:], lgin[:], AF.Exp, bias=0.0, scale=1.0,
                                     accum_out=esum2[:])
                rinv2 = mp.tile([TB, 1], F32, tag="rinv2")
                nc.vector.reciprocal(rinv2[:], esum2[:])
                rw_sb = mp.tile([TB, E], F32, tag="rw_sb")
                nc.vector.tensor_scalar_mul(rw_sb[:], lg[:], rinv2[:])
                # top-2 mask + renormalize
                m1 = mp.tile([TB, 1], F32, tag="m1")
                nc.vector.tensor_reduce(m1[:], rw_sb[:], axis=AX.X, op=ALU.max)
                e1 = mp.tile([TB, E], F32, tag="e1")
                nc.vector.tensor_scalar(e1[:], rw_sb[:], m1[:], None, op0=ALU.is_equal)
                e1s = mp.tile([TB, E], F32, tag="e1s")
                nc.vector.tensor_scalar_mul(e1s[:], e1[:], 2.0)
                msk2 = mp.tile([TB, E], F32, tag="msk2")
                nc.vector.tensor_sub(msk2[:], rw_sb[:], e1s[:])
                m2 = mp.tile([TB, 1], F32, tag="m2")
                nc.vector.tensor_reduce(m2[:], msk2[:], axis=AX.X, op=ALU.max)
                e2 = mp.tile([TB, E], F32, tag="e2")
                nc.vector.tensor_scalar(e2[:], msk2[:], m2[:], None, op0=ALU.is_equal)
                emask = mp.tile([TB, E], F32, tag="emask")
                nc.vector.tensor_add(emask[:], e1[:], e2[:])
                den = mp.tile([TB, 1], F32, tag="den")
                nc.vector.tensor_add(den[:], m1[:], m2[:])
                dinv = mp.tile([TB, 1], F32, tag="dinv")
                nc.vector.reciprocal(dinv[:], den[:])
                wte = mp.tile([TB, E], F32, tag="wte")
                nc.vector.tensor_mul(wte[:], rw_sb[:], emask[:])
                nc.vector.tensor_scalar_mul(wte[:], wte[:], dinv[:])
                xpa = mp.tile([TB, E + 1024], BF16, tag="xpa")
                nc.vector.tensor_copy(xpa[:, 0:E], wte[:])
                nc.vector.tensor_copy(xpa[:, E:E + 1024], xm[:, 0:1024])
                nc.sync.dma_start(out=ag_xa_in[:, :], in_=xpa[:])
                nc.gpsimd.collective_compute(
                    "AllGather", ALU.bypass, replica_groups=rg,
                    ins=[ag_xa_in[:, :].opt()], outs=[ag_xa_out[:, :, :].opt()],
                )
                xpb = mp.tile([TB, 1024], BF16, tag="xpb")
                nc.vector.tensor_copy(xpb[:], xm[:, 1024:2048])
                nc.sync.dma_start(out=ag_xb_in[:, :], in_=xpb[:])
                nc.gpsimd.collective_compute(
                    "AllGather", ALU.bypass, replica_groups=rg,
                    ins=[ag_xb_in[:, :].opt()], outs=[ag_xb_out[:, :, :].opt()],
                )

            # =============== MoE expert compute (capacity gather) ===============
            with (
                tc.tile_pool(name="moe1", bufs=1) as m1p,
                tc.tile_pool(name="moew", bufs=6) as wp,
                tc.tile_pool(name="moed", bufs=3) as dp,
                tc.tile_pool(name="moet", bufs=2) as tp,
            ):
                # w_te columns first (small), so routing/dispatch overlaps
                # the big x_all DMA
                wcols = m1p.tile([128, NCORES, E], BF16, tag="wcols")
                for b in range(NCORES):
                    nc.sync.dma_start(out=wcols[:, b, :],
                                      in_=ag_xa_out[b, :, 0:E])
                # all tokens, token-major: [t, b, x] in two halves
                x_alla = m1p.tile([128, NCORES, 1024], BF16, tag="x_alla")
                for b in range(NCORES):
                    nc.sync.dma_start(out=x_alla[:, b, :],
                                      in_=ag_xa_out[b, :, E:E + 1024])
                x_allb = m1p.tile([128, NCORES, 1024], BF16, tag="x_allb")
                for b in range(NCORES):
                    nc.sync.dma_start(out=x_allb[:, b, :], in_=ag_xb_out[b, :, :])
                # --- routing indicator: is this core's expert in token's top-2 ---
                woc = m1p.tile([128, NCORES], F32, tag="woc")
                for b in range(NCORES):
                    t8 = tp.tile([128, E], F32, tag="t8")
                    nc.vector.tensor_mul(t8[:], wcols[:, b, :], esel_sb[:])
                    nc.vector.tensor_reduce(woc[:, b:b + 1], t8[:], axis=AX.X,
                                            op=ALU.add)
                ind_bf = m1p.tile([128, NCORES], BF16, tag="ind_bf")
                nc.vector.tensor_scalar(ind_bf[:], woc[:], 0.0, None,
                                        op0=ALU.is_gt)

                # --- capacity slot per routed token (exclusive running count) ---
                plc = psC.tile([128, 512], F32, tag="mid")
                nc.tensor.matmul(plc[:, 0:NCORES], ltri_sb[:], ind_bf[:],
                                 start=True, stop=True)
                posL = m1p.tile([128, NCORES], F32, tag="posL")
                nc.scalar.copy(posL[:], plc[:, 0:NCORES])
                ptc = psC.tile([128, 512], F32, tag="mid")
                nc.tensor.matmul(ptc[0:1, 0:NCORES], onesc_sb[:], ind_bf[:],
                                 start=True, stop=True)
                totf = m1p.tile([1, NCORES], F32, tag="totf")
                nc.scalar.copy(totf[:], ptc[0:1, 0:NCORES])
                # exclusive cumsum over the 8 chunk totals
                c1 = m1p.tile([1, NCORES], F32, tag="c1")
                nc.vector.memset(c1[:], 0.0)
                nc.vector.tensor_copy(c1[:, 1:8], totf[:, 0:7])
                c2 = m1p.tile([1, NCORES], F32, tag="c2")
                nc.vector.tensor_copy(c2[:, 0:1], c1[:, 0:1])
                nc.vector.tensor_add(c2[:, 1:8], c1[:, 1:8], c1[:, 0:7])
                c3 = m1p.tile([1, NCORES], F32, tag="c3")
                nc.vector.tensor_copy(c3[:, 0:2], c2[:, 0:2])
                nc.vector.tensor_add(c3[:, 2:8], c2[:, 2:8], c2[:, 0:6])
                c4 = m1p.tile([1, NCORES], F32, tag="c4")
                nc.vector.tensor_copy(c4[:, 0:4], c3[:, 0:4])
                nc.vector.tensor_add(c4[:, 4:8], c3[:, 4:8], c3[:, 0:4])
                poc = psC.tile([128, 512], F32, tag="mid")
                nc.tensor.matmul(poc[:, 0:NCORES], ones1f_sb[:], c4[:],
                                 start=True, stop=True)
                pos2 = m1p.tile([128, NCORES], F32, tag="pos2")
                nc.vector.tensor_add(pos2[:], posL[:], poc[:, 0:NCORES])
                nbig = m1p.tile([128, NCORES], F32, tag="nbig")
                nc.vector.tensor_scalar(nbig[:], ind_bf[:], -4096.0, 4096.0,
                                        op0=ALU.mult, op1=ALU.add)
                nc.vector.tensor_add(pos2[:], pos2[:], nbig[:])

                # --- one-hot dispatch P[t, c] and its transpose ---
                P = m1p.tile([128, NCORES, CAP], BF16, tag="P")
                for b in range(NCORES):
                    nc.vector.tensor_scalar(P[:, b, :], iotac_sb[:],
                                            pos2[:, b:b + 1], None,
                                            op0=ALU.is_equal)
                PT = m1p.tile([128, CB, S], BF16, tag="PT")
                for b in range(NCORES):
                    for j in range(CB):
                        ptb = psD.tile([128, 128], BF16, tag="midb")
                        nc.tensor.transpose(ptb[:], P[:, b, j * 128:(j + 1) * 128],
                                            identb[:])
                        nc.scalar.copy(PT[:, j, b * 128:(b + 1) * 128], ptb[:])

                # --- gather xg[h, c] = x^T P and this expert's weights ---
                xg = m1p.tile([128, HC, CAP], BF16, tag="xg")
                for kc in range(HC):
                    xa = x_alla if kc < 8 else x_allb
                    k0 = (kc % 8) * 128
                    gp = psC.tile([128, 512], F32, tag="mid")
                    for b in range(NCORES):
                        nc.tensor.matmul(gp[:, 0:CAP],
                                         xa[:, b, k0:k0 + 128],
                                         P[:, b, :], start=(b == 0),
                                         stop=(b == NCORES - 1))
                    nc.scalar.copy(xg[:, kc, :], gp[:, 0:CAP])
                wgp = psC.tile([128, 512], F32, tag="mid")
                for b in range(NCORES):
                    nc.tensor.matmul(wgp[0:E, 0:CAP], wcols[:, b, :],
                                     P[:, b, :], start=(b == 0),
                                     stop=(b == NCORES - 1))
                wg8 = m1p.tile([E, CAP], BF16, tag="wg8")
                nc.scalar.copy(wg8[:], wgp[0:E, 0:CAP])
                wbp = psC.tile([128, 512], F32, tag="mid")
                nc.tensor.matmul(wbp[:, 0:CAP], sel_sb[:], wg8[:],
                                 start=True, stop=True)
                wbc = m1p.tile([128, CAP], F32, tag="wbc")
                nc.scalar.copy(wbc[:], wbp[:, 0:CAP])

                # --- up / gate / silu over gathered tokens ---
                intert = m1p.tile([128, FT, CAP], BF16, tag="intert")
                for ft in range(FT):
                    ut = wp.tile([128, HC, 128], BF16, tag="w_up")
                    nc.scalar.dma_start(out=ut[:], in_=upw[ft, :, :, :])
                    gt = wp.tile([128, HC, 128], BF16, tag="w_up")
                    nc.scalar.dma_start(out=gt[:], in_=gatew[ft, :, :, :])
                    big = psA.tile([TB, S], F32, tag="big")
                    pu = big[:, 0:CAP]
                    pg = big[:, 512:512 + CAP]
                    for kc in range(HC):
                        nc.tensor.matmul(pu, ut[:, kc, :], xg[:, kc, :],
                                         start=(kc == 0), stop=(kc == HC - 1))
                    for kc in range(HC):
                        nc.tensor.matmul(pg, gt[:, kc, :], xg[:, kc, :],
                                         start=(kc == 0), stop=(kc == HC - 1))
                    sl = tp.tile([128, CAP], F32, tag="silu_t")
                    nc.scalar.activation(sl[:], pu, AF.Silu)
                    nc.vector.tensor_mul(sl[:], sl[:], pg)
                    nc.vector.tensor_mul(intert[:, ft, :], sl[:], wbc[:])

                # --- down proj (ygT[c, h]) + scatter + chunked ReduceScatter,
                #     pipelined over four 512-wide h quarters; each quarter's
                #     residual-add + output store rides behind its RS ---
                out_sb = m1p.tile([TB, H], F32, tag="out_sb")
                ygT = m1p.tile([128, CB, 512], BF16, tag="ygT")
                for hq in range(4):
                    for sub in range(512 // DH):
                        h0 = hq * 512 + sub * DH
                        dw = dp.tile([128, FT, DH], BF16, tag="w_dn")
                        nc.scalar.dma_start(
                            out=dw[:],
                            in_=downw[:, :, h0:h0 + DH].rearrange("f p h -> p f h"))
                        for j in range(CB):
                            big = psA.tile([TB, S], F32, tag="big")
                            pd = big[:, 0:DH]
                            for ft in range(FT):
                                nc.tensor.matmul(
                                    pd, intert[:, ft, j * 128:(j + 1) * 128],
                                    dw[:, ft, :], start=(ft == 0),
                                    stop=(ft == FT - 1))
                            nc.scalar.copy(ygT[:, j, sub * DH:sub * DH + DH], pd)
                    # scatter this quarter for every token chunk, then RS it
                    for b in range(NCORES):
                        ys = tp.tile([128, 512], BF16, tag="y_sb")
                        big = psA.tile([TB, S], F32, tag="big")
                        ps = big[:, 0:512]
                        for j in range(CB):
                            nc.tensor.matmul(ps, PT[:, j, b * 128:(b + 1) * 128],
                                             ygT[:, j, :],
                                             start=(j == 0), stop=(j == CB - 1))
                        nc.vector.tensor_copy(ys[:], ps)
                        nc.sync.dma_start(out=y_part[hq][b, :, :], in_=ys[:])
                    nc.gpsimd.collective_compute(
                        "ReduceScatter", ALU.add, replica_groups=rg,
                        ins=[y_part[hq][:, :, :].opt()],
                        outs=[y_rs[hq][:, :].opt()],
                    )
                    # final out = x2 + y for this quarter, overlapping the
                    # next quarter's compute
                    yq = tp.tile([TB, 512], BF16, tag="yq")
                    nc.sync.dma_start(out=yq[:], in_=y_rs[hq][:, :])
                    nc.vector.tensor_add(out_sb[:, hq * 512:(hq + 1) * 512],
                                         x2[:, hq * 512:(hq + 1) * 512], yq[:])
                    nc.sync.dma_start(out=out_ext[:, hq * 512:(hq + 1) * 512],
                                      in_=out_sb[:, hq * 512:(hq + 1) * 512])

    nc.finalize()
    return nc


_NC_CACHE = None


def kernel(**inputs) -> np.ndarray:
    global _NC_CACHE
    hidden = np.asarray(inputs["hidden_states"], np.float32).reshape(S, H)
    cos = np.asarray(inputs["cos"], np.float32).reshape(S, HD)
    sin = np.asarray(inputs["sin"], np.float32).reshape(S, HD)
    q_w = np.asarray(inputs["q_w"], np.float32)
    k_w = np.asarray(inputs["k_w"], np.float32)
    v_w = np.asarray(inputs["v_w"], np.float32)
    o_w = np.asarray(inputs["o_w"], np.float32)
    ln1 = np.asarray(inputs["ln1_w"], np.float32)
    ln2 = np.asarray(inputs["ln2_w"], np.float32)
    router_w = np.asarray(inputs["router_w"], np.float32)
    up_w = np.asarray(inputs["up_w"], np.float32)
    gate_w = np.asarray(inputs["gate_w"], np.float32)
    down_w = np.asarray(inputs["down_w"], np.float32)

    scale = HD ** -0.5
    BF = ml_dtypes.bfloat16
    ident = np.eye(128, dtype=np.float32)
    identb = np.eye(128, dtype=BF)
    ltri = (np.arange(128)[:, None] < np.arange(128)[None, :]).astype(BF)
    onesc = np.ones((128, 1), BF)
    ones1f = np.ones((1, 128), np.float32)
    iotac = np.broadcast_to(np.arange(CAP, dtype=np.float32), (128, CAP)).copy()
    def retile_w(w):
        d = w.shape[1]
        return np.ascontiguousarray(
            w.reshape(HC, 128, d // 512, 512).transpose(2, 1, 0, 3).astype(BF))

    qw_f = retile_w(ln1[:, None] * q_w)
    kw_f = retile_w(ln1[:, None] * k_w)
    vw_f = retile_w(ln1[:, None] * v_w)
    ow_f = retile_w(o_w)
    rw_f = np.ascontiguousarray(ln2[:, None] * router_w)

    tri = np.where(np.arange(TB)[None, :] <= np.arange(TB)[:, None], 0.0,
                   NEG).astype(np.float32)

    if _NC_CACHE is None:
        _NC_CACHE = build_nc()
    nc = _NC_CACHE

    in_maps = []
    for c in range(NCORES):
        t0 = c * TB
        cos_c = cos[t0:t0 + TB]
        sin_c = sin[t0:t0 + TB]
        bias_arr = np.zeros((NCORES, TB, TB), np.float32)
        for b in range(NCORES):
            if b == c:
                bias_arr[b] = tri
            elif b > c:
                bias_arr[b] = NEG
        sel = np.zeros((E, 128), BF)
        sel[c, :] = 1.0
        esel = np.zeros((128, E), BF)
        esel[:, c] = 1.0
        upw_t = np.ascontiguousarray(
            (ln2[:, None] * up_w[c]).reshape(HC, 128, FT, 128)
            .transpose(2, 1, 0, 3).astype(BF))
        gatew_t = np.ascontiguousarray(
            (ln2[:, None] * gate_w[c]).reshape(HC, 128, FT, 128)
            .transpose(2, 1, 0, 3).astype(BF))
        downw_t = np.ascontiguousarray(down_w[c].reshape(FT, 128, H).astype(BF))
        in_maps.append({
            "h": np.ascontiguousarray(hidden[t0:t0 + TB]),
            "cos_q": np.ascontiguousarray(np.tile(cos_c, (1, NH)) * scale),
            "sin_q": np.ascontiguousarray(np.tile(sin_c, (1, NH)) * scale),
            "cos_k": np.ascontiguousarray(np.tile(cos_c, (1, KVH))),
            "sin_k": np.ascontiguousarray(np.tile(sin_c, (1, KVH))),
            "bias_all": bias_arr,
            "ident": ident,
            "identb": identb,
            "sel": sel,
            "esel": esel,
            "ltri": ltri,
            "onesc": onesc,
            "ones1f": ones1f,
            "iotac": iotac,
            "qw": qw_f, "kw": kw_f, "vw": vw_f, "ow": ow_f, "rw": rw_f,
            "upw": upw_t, "gatew": gatew_t, "downw": downw_t,
        })

    trace = os.environ.get("KERNEL_TRACE", "0") == "1"
    res = run_bass_kernel_spmd(nc, in_maps, core_ids=list(range(NCORES)), trace=trace)
    kernel.last_result = res
    out = np.concatenate([res.results[c]["out"] for c in range(NCORES)], axis=0)
    return out.reshape(B, S, H).astype(np.float32)



# revision 70
# speedup vs baseline: 1.0266x; 1.0266x over previous
"""Mixtral decoder layer on 8 TRN2 NeuronCores.

Sharding:
  - Attention: sequence-parallel. Core c owns tokens [c*128, (c+1)*128).
    Each core computes rmsnorm1 stats + q/k/v projections (1/rms folded
    into the PSUM copy-out) + RoPE for its own 128 tokens, AllGathers
    K (pre-transposed) | V in bf16, then computes causal attention +
    o-projection + residual for its token block.
  - MoE: expert-parallel with capacity-384 token gather. Core c owns
    expert c. After rmsnorm2 + router (top-2 weights per token), the
    normed activations + combine weights are AllGathered in bf16 (two
    halves, so dispatch overlaps the second). Each core builds a
    one-hot dispatch matrix P[t, c] from a cumulative-count matmul,
    gathers its <=384 routed tokens via PE matmuls, runs
    up/gate/silu/down on the compressed block, scatters y = P @ ygT
    back to [t, H], and a ReduceScatter(add) chunked over four h
    quarters (pipelined with down/scatter compute) returns each core
    its token block of the sum.
  - All heavy matmuls run in bf16 (same PE rate as fp32r here, but
    half the weight DMA); router stays fp32r for exact top-2.
  - ln1_w / ln2_w are folded into downstream weight matrices on host.

Self-contained: hardcodes all shapes from the problem spec.
"""
import os

import ml_dtypes
import numpy as np

import concourse.bass as bass  # noqa: F401
import concourse.mybir as mybir
from concourse import bacc, tile
from concourse.bass_utils import run_bass_kernel_spmd

F32 = mybir.dt.float32
F32R = mybir.dt.float32r
BF16 = mybir.dt.bfloat16
AF = mybir.ActivationFunctionType
ALU = mybir.AluOpType
AX = mybir.AxisListType

NCORES = 8
B, S, H = 1, 1024, 2048
NH, KVH, HD = 16, 4, 128
E, TOPK, F = 8, 2, 4096
EPS = 1e-6
TB = S // NCORES          # tokens per core = 128
HC = H // 128             # 16 contraction chunks over H
FT = F // 128             # 32 F tiles
NEG = -1.0e30
CAP = 384                 # expert token capacity (mean load 256, ~9 sigma)
CB = CAP // 128           # capacity chunks of 128
DH = 256                  # down-proj h-slice per weight load


def build_nc():
    nc = bacc.Bacc(num_devices=NCORES)

    # ---- per-core external inputs ----
    h_in = nc.dram_tensor("h", [TB, H], F32, kind="ExternalInput")
    cos_q = nc.dram_tensor("cos_q", [TB, H], F32, kind="ExternalInput")
    sin_q = nc.dram_tensor("sin_q", [TB, H], F32, kind="ExternalInput")
    cos_k = nc.dram_tensor("cos_k", [TB, KVH * HD], F32, kind="ExternalInput")
    sin_k = nc.dram_tensor("sin_k", [TB, KVH * HD], F32, kind="ExternalInput")
    bias_all = nc.dram_tensor("bias_all", [NCORES, TB, TB], F32, kind="ExternalInput")
    ident_in = nc.dram_tensor("ident", [128, 128], F32, kind="ExternalInput")
    identb_in = nc.dram_tensor("identb", [128, 128], BF16, kind="ExternalInput")
    sel_in = nc.dram_tensor("sel", [E, 128], BF16, kind="ExternalInput")
    esel_in = nc.dram_tensor("esel", [128, E], BF16, kind="ExternalInput")
    ltri_in = nc.dram_tensor("ltri", [128, 128], BF16, kind="ExternalInput")
    onesc_in = nc.dram_tensor("onesc", [128, 1], BF16, kind="ExternalInput")
    ones1f_in = nc.dram_tensor("ones1f", [1, 128], F32, kind="ExternalInput")
    iotac_in = nc.dram_tensor("iotac", [128, CAP], F32, kind="ExternalInput")
    qw = nc.dram_tensor("qw", [4, 128, HC, 512], BF16, kind="ExternalInput")
    kw = nc.dram_tensor("kw", [1, 128, HC, 512], BF16, kind="ExternalInput")
    vw = nc.dram_tensor("vw", [1, 128, HC, 512], BF16, kind="ExternalInput")
    ow = nc.dram_tensor("ow", [4, 128, HC, 512], BF16, kind="ExternalInput")
    rw_in = nc.dram_tensor("rw", [H, E], F32, kind="ExternalInput")
    # expert weights, host-retiled:
    #   upw/gatew: [FT, 128(p=H row in chunk), HC, 128(f)]
    #   downw:     [FT, 128(p=F row in tile), H]
    upw = nc.dram_tensor("upw", [FT, 128, HC, 128], BF16, kind="ExternalInput")
    gatew = nc.dram_tensor("gatew", [FT, 128, HC, 128], BF16, kind="ExternalInput")
    downw = nc.dram_tensor("downw", [FT, 128, H], BF16, kind="ExternalInput")

    out_ext = nc.dram_tensor("out", [TB, H], F32, kind="ExternalOutput")

    # ---- internal DRAM (collective bounce buffers) ----
    ag_kv_in = nc.dram_tensor("ag_kv_in", [128, 1024], BF16)
    ag_kv_out = nc.dram_tensor("ag_kv_out", [NCORES, 128, 1024], BF16,
                               addr_space="Shared")
    ag_xa_in = nc.dram_tensor("ag_xa_in", [TB, E + 1024], BF16)
    ag_xa_out = nc.dram_tensor("ag_xa_out", [NCORES, TB, E + 1024], BF16,
                               addr_space="Shared")
    ag_xb_in = nc.dram_tensor("ag_xb_in", [TB, 1024], BF16)
    ag_xb_out = nc.dram_tensor("ag_xb_out", [NCORES, TB, 1024], BF16,
                               addr_space="Shared")
    y_part = [nc.dram_tensor(f"y_part{i}", [NCORES, TB, 512], BF16) for i in range(4)]
    y_rs = [nc.dram_tensor(f"y_rs{i}", [TB, 512], BF16) for i in range(4)]

    rg = [list(range(NCORES))]

    with tile.TileContext(nc) as tc:
        with (
            tc.tile_pool(name="glob", bufs=1) as glob,
            tc.tile_pool(name="psA", bufs=2, space="PSUM") as psA,
            tc.tile_pool(name="psC", bufs=2, space="PSUM") as psC,
            tc.tile_pool(name="psD", bufs=2, space="PSUM") as psD,
        ):
            h_sb = glob.tile([TB, H], F32, tag="h_sb")
            nc.sync.dma_start(out=h_sb[:], in_=h_in[:, :])
            ident = glob.tile([128, 128], F32, tag="ident")
            nc.sync.dma_start(out=ident[:], in_=ident_in[:, :])
            identb = glob.tile([128, 128], BF16, tag="identb")
            nc.sync.dma_start(out=identb[:], in_=identb_in[:, :])
            x2 = glob.tile([TB, H], F32, tag="x2")
            epsc = glob.tile([TB, 1], F32, tag="epsc")
            nc.vector.memset(epsc[:], EPS)
            # MoE dispatch constants, preloaded so nothing queues behind
            # the big activation DMAs later
            sel_sb = glob.tile([E, 128], BF16, tag="sel_sb")
            nc.sync.dma_start(out=sel_sb[:], in_=sel_in[:, :])
            esel_sb = glob.tile([128, E], BF16, tag="esel_sb")
            nc.sync.dma_start(out=esel_sb[:], in_=esel_in[:, :])
            ltri_sb = glob.tile([128, 128], BF16, tag="ltri_sb")
            nc.sync.dma_start(out=ltri_sb[:], in_=ltri_in[:, :])
            onesc_sb = glob.tile([128, 1], BF16, tag="onesc_sb")
            nc.sync.dma_start(out=onesc_sb[:], in_=onesc_in[:, :])
            ones1f_sb = glob.tile([1, 128], F32, tag="ones1f_sb")
            nc.sync.dma_start(out=ones1f_sb[:], in_=ones1f_in[:, :])
            iotac_sb = glob.tile([128, CAP], F32, tag="iotac_sb")
            nc.sync.dma_start(out=iotac_sb[:], in_=iotac_in[:, :])

            # =============== attention ===============
            with tc.tile_pool(name="at_keep", bufs=1) as akp:
                qr = akp.tile([TB, NH, HD], F32, tag="qr")
                kv_loc = akp.tile([TB, 1024], F32, tag="kv_loc")  # [k | v]

                with (
                    tc.tile_pool(name="at_pre", bufs=1) as pp1,
                    tc.tile_pool(name="at_pre2", bufs=3) as pp2,
                ):
                    # --- rmsnorm1 stats (scale applied at proj copy-out) ---
                    sq = pp1.tile([TB, H], F32, tag="sq")
                    nc.vector.tensor_mul(sq[:], h_sb[:], h_sb[:])
                    var = pp1.tile([TB, 1], F32, tag="var")
                    nc.vector.tensor_reduce(var[:], sq[:], axis=AX.X, op=ALU.add)
                    sd = pp1.tile([TB, 1], F32, tag="sd")
                    nc.scalar.activation(sd[:], var[:], AF.Sqrt, bias=epsc[:], scale=1.0 / H)
                    rs1 = pp1.tile([TB, 1], F32, tag="rs1")
                    nc.vector.reciprocal(rs1[:], sd[:])

                    # --- hT (16 PE transposes of the raw residual) ---
                    x1t = pp1.tile([128, HC, TB], BF16, tag="x1t")
                    for kc in range(HC):
                        pt = psC.tile([128, 128], F32, tag="mid")
                        nc.tensor.transpose(pt[:], h_sb[:, kc * 128:(kc + 1) * 128], ident[:])
                        nc.scalar.copy(x1t[:, kc, :], pt[:])

                    # --- q/k/v projections (out = [tok, dim], scaled by 1/rms) ---
                    q_sb = pp1.tile([TB, NH * HD], F32, tag="q_sb")

                    def proj(w_dram, n_dim, out_fn):
                        for n0 in range(0, n_dim, 512):
                            pp = psC.tile([128, 512], F32, tag="mid")
                            wt = pp2.tile([128, HC, 512], BF16, tag="w_sb")
                            nc.scalar.dma_start(
                                out=wt[:],
                                in_=w_dram[n0 // 512, :, :, :],
                            )
                            for kc in range(HC):
                                nc.tensor.matmul(
                                    pp[:], x1t[:, kc, :], wt[:, kc, :],
                                    start=(kc == 0), stop=(kc == HC - 1),
                                )
                            out_fn(n0, pp[:])

                    def rope(src3, cos3, sin3, dst3, nh):
                        hh = HD // 2
                        a = pp2.tile([TB, NH, hh], F32, tag="rope_t")
                        b2 = pp2.tile([TB, NH, hh], F32, tag="rope_t")
                        nc.vector.tensor_mul(a[:, 0:nh, :], src3[:, :, 0:hh], cos3[:, :, 0:hh])
                        nc.vector.tensor_mul(b2[:, 0:nh, :], src3[:, :, hh:], sin3[:, :, 0:hh])
                        nc.vector.tensor_sub(dst3[:, :, 0:hh], a[:, 0:nh, :], b2[:, 0:nh, :])
                        c2 = pp2.tile([TB, NH, hh], F32, tag="rope_t")
                        d2 = pp2.tile([TB, NH, hh], F32, tag="rope_t")
                        nc.vector.tensor_mul(c2[:, 0:nh, :], src3[:, :, hh:], cos3[:, :, hh:])
                        nc.vector.tensor_mul(d2[:, 0:nh, :], src3[:, :, 0:hh], sin3[:, :, hh:])
                        nc.vector.tensor_add(dst3[:, :, hh:], c2[:, 0:nh, :], d2[:, 0:nh, :])

                    # v then k, so the combined AllGather launches before q
                    proj(vw, KVH * HD,
                         lambda n0, pp: nc.vector.tensor_scalar_mul(
                             kv_loc[:, 512:1024], pp, rs1[:]))
                    kvb = pp1.tile([128, 1024], BF16, tag="kvb")
                    nc.vector.tensor_copy(kvb[:, 512:1024], kv_loc[:, 512:1024])

                    proj(kw, KVH * HD,
                         lambda n0, pp: nc.vector.tensor_scalar_mul(
                             kv_loc[:, 0:512], pp, rs1[:]))
                    ck = pp1.tile([TB, KVH, HD], F32, tag="ck")
                    skv = pp1.tile([TB, KVH, HD], F32, tag="skv")
                    nc.sync.dma_start(out=ck[:], in_=cos_k[:, :].rearrange("t (h d) -> t h d", d=HD))
                    nc.sync.dma_start(out=skv[:], in_=sin_k[:, :].rearrange("t (h d) -> t h d", d=HD))
                    kr = pp1.tile([TB, KVH, HD], F32, tag="kr")
                    rope(kv_loc[:, 0:512].rearrange("t (h d) -> t h d", d=HD), ck, skv, kr[:], KVH)

                    # transpose local K now so receivers skip it: payload
                    # cols g*128:(g+1)*128 hold kT[hd, tok] for group g
                    for g in range(KVH):
                        ptk = psC.tile([128, 512], F32, tag="mid")
                        nc.tensor.transpose(ptk[:, 0:128], kr[:, g, :], ident[:])
                        nc.vector.tensor_copy(kvb[:, g * 128:(g + 1) * 128],
                                              ptk[:, 0:128])
                    nc.sync.dma_start(out=ag_kv_in[:, :], in_=kvb[:])
                    nc.gpsimd.collective_compute(
                        "AllGather", ALU.bypass, replica_groups=rg,
                        ins=[ag_kv_in[:, :].opt()], outs=[ag_kv_out[:, :, :].opt()],
                    )

                    # --- q-proj + RoPE overlap the kv AllGather ---
                    proj(qw, NH * HD,
                         lambda n0, pp: nc.vector.tensor_scalar_mul(
                             q_sb[:, n0:n0 + 512], pp, rs1[:]))
                    cq = pp1.tile([TB, NH, HD], F32, tag="cq")
                    sqv = pp1.tile([TB, NH, HD], F32, tag="sqv")
                    nc.sync.dma_start(out=cq[:], in_=cos_q[:, :].rearrange("t (h d) -> t h d", d=HD))
                    nc.sync.dma_start(out=sqv[:], in_=sin_q[:, :].rearrange("t (h d) -> t h d", d=HD))
                    rope(q_sb[:].rearrange("t (h d) -> t h d", d=HD), cq, sqv, qr[:], NH)

                # --- attention proper ---
                with (
                    tc.tile_pool(name="at_core", bufs=1) as acp,
                    tc.tile_pool(name="at_core2", bufs=2) as acp2,
                ):
                    qt = acp.tile([128, NH, TB], BF16, tag="qt")
                    for hh in range(NH):
                        pt = psC.tile([128, 128], F32, tag="mid")
                        nc.tensor.transpose(pt[:], qr[:, hh, :], ident[:])
                        nc.scalar.copy(qt[:, hh, :], pt[:])

                    bias_sb = acp.tile([TB, NCORES, TB], F32, tag="bias_sb")
                    nc.sync.dma_start(out=bias_sb[:],
                                      in_=bias_all[:, :, :].rearrange("b q k -> q b k"))
                    kt = acp.tile([128, KVH, S], BF16, tag="kt")  # [hd, g, keys]
                    for g in range(KVH):
                        nc.sync.dma_start(
                            out=kt[:, g, :].rearrange("p (b t) -> p b t", t=TB),
                            in_=ag_kv_out[:, :, g * 128:(g + 1) * 128]
                                .rearrange("b p t -> p b t"))
                    v_sb = acp.tile([TB, NCORES, 512], BF16, tag="v_sb")
                    for b in range(NCORES):
                        nc.sync.dma_start(out=v_sb[:, b, :],
                                          in_=ag_kv_out[b, :, 512:1024])

                    attn_ot = acp.tile([128, NH, TB], BF16, tag="attn_ot")  # [hd, head, tok]
                    for hh in range(NH):
                        g = hh // (NH // KVH)
                        ps = psA.tile([TB, S], F32, tag="big")
                        for n0 in range(0, S, 512):
                            nc.tensor.matmul(ps[:, n0:n0 + 512], qt[:, hh, :],
                                             kt[:, g, n0:n0 + 512], start=True,
                                             stop=True)
                        sc_sb = acp2.tile([TB, NCORES, TB], F32, tag="sc_sb")
                        nc.vector.tensor_add(sc_sb[:],
                                             ps[:].rearrange("q (b k) -> q b k", k=TB),
                                             bias_sb[:])
                        flat = sc_sb[:].rearrange("q b k -> q (b k)")
                        esum = acp2.tile([TB, 1], F32, tag="esum")
                        nc.scalar.activation(flat, flat, AF.Exp, bias=0.0, scale=1.0,
                                             accum_out=esum[:])
                        rinv = acp2.tile([TB, 1], F32, tag="rinv")
                        nc.vector.reciprocal(rinv[:], esum[:])
                        prb = acp2.tile([TB, NCORES, TB], BF16, tag="prb")
                        nc.vector.tensor_scalar_mul(
                            prb[:].rearrange("q b k -> q (b k)"), flat, rinv[:])

                        pavt = psC.tile([128, 512], F32, tag="mid")
                        pav = pavt[:, 0:TB]
                        for b in range(NCORES):
                            pt = psD.tile([128, 128], BF16, tag="midb")
                            nc.tensor.transpose(pt[:], prb[:, b, :], identb[:])
                            at_sb = acp2.tile([TB, TB], BF16, tag="at_sb")
                            nc.vector.tensor_copy(at_sb[:], pt[:])
                            nc.tensor.matmul(pav[:],
                                             v_sb[:, b, g * 128:(g + 1) * 128],
                                             at_sb[:], start=(b == 0),
                                             stop=(b == NCORES - 1))
                        nc.scalar.copy(attn_ot[:, hh, :], pav)

                    # --- o projection + residual ---
                    for n0 in range(0, H, 512):
                        po = psC.tile([128, 512], F32, tag="mid")
                        wt = acp2.tile([128, HC, 512], BF16, tag="w_sb2")
                        nc.scalar.dma_start(
                            out=wt[:],
                            in_=ow[n0 // 512, :, :, :])
                        for kc in range(HC):
                            nc.tensor.matmul(po[:], attn_ot[:, kc, :], wt[:, kc, :],
                                             start=(kc == 0), stop=(kc == HC - 1))
                        nc.vector.tensor_add(x2[:, n0:n0 + 512], h_sb[:, n0:n0 + 512], po[:])

            # =============== rmsnorm2 + router + AG ===============
            with tc.tile_pool(name="mid", bufs=1) as mp:
                # transposes of the raw x2 start immediately on the PE;
                # the rmsnorm stats run concurrently on vector/scalar
                xmt = mp.tile([128, HC, TB], F32R, tag="xmt")
                for kc in range(HC):
                    pt = psC.tile([128, 128], F32, tag="mid")
                    nc.tensor.transpose(pt[:], x2[:, kc * 128:(kc + 1) * 128], ident[:])
                    nc.scalar.copy(xmt[:, kc, :], pt[:])

                sq2 = mp.tile([TB, H], F32, tag="sq2")
                nc.vector.tensor_mul(sq2[:], x2[:], x2[:])
                var2 = mp.tile([TB, 1], F32, tag="var2")
                nc.vector.tensor_reduce(var2[:], sq2[:], axis=AX.X, op=ALU.add)
                sd2 = mp.tile([TB, 1], F32, tag="sd2")
                nc.scalar.activation(sd2[:], var2[:], AF.Sqrt, bias=epsc[:], scale=1.0 / H)
                rs2 = mp.tile([TB, 1], F32, tag="rs2")
                nc.vector.reciprocal(rs2[:], sd2[:])
                xm = mp.tile([TB, H], F32, tag="xm")
                nc.vector.tensor_scalar_mul(xm[:], x2[:], rs2[:])

                # router (ln2 folded into rw on host); logits scaled by 1/rms
                rwt = mp.tile([128, HC, E], F32R, tag="rwt")
                nc.sync.dma_start(out=rwt[:],
                                  in_=rw_in[:, :].rearrange("(k p) e -> p k e",
                                                            p=128).bitcast(F32R))
                plt = psC.tile([128, 512], F32, tag="mid")
                pl = plt[0:TB, 0:E]
                for kc in range(HC):
                    nc.tensor.matmul(pl[:], xmt[:, kc, :], rwt[:, kc, :],
                                     start=(kc == 0), stop=(kc == HC - 1))
                lgin = mp.tile([TB, E], F32, tag="lgin")
                nc.vector.tensor_scalar_mul(lgin[:], pl[:], rs2[:])
                lg = mp.tile([TB, E], F32, tag="lg")
                esum2 = mp.tile([TB, 1], F32, tag="esum2")
                nc.scalar.activation(lg[:], lgin[:], AF.Exp, bias=0.0, scale=1.0,
                                     accum_out=esum2[:])
                rinv2 = mp.tile([TB, 1], F32, tag="rinv2")
                nc.vector.reciprocal(rinv2[:], esum2[:])
                rw_sb = mp.tile([TB, E], F32, tag="rw_sb")
                nc.vector.tensor_scalar_mul(rw_sb[:], lg[:], rinv2[:])
                # top-2 mask + renormalize
                m1 = mp.tile([TB, 1], F32, tag="m1")
                nc.vector.tensor_reduce(m1[:], rw_sb[:], axis=AX.X, op=ALU.max)
                e1 = mp.tile([TB, E], F32, tag="e1")
                nc.vector.tensor_scalar(e1[:], rw_sb[:], m1[:], None, op0=ALU.is_equal)
                e1s = mp.tile([TB, E], F32, tag="e1s")
                nc.vector.tensor_scalar_mul(e1s[:], e1[:], 2.0)
                msk2 = mp.tile([TB, E], F32, tag="msk2")
                nc.vector.tensor_sub(msk2[:], rw_sb[:], e1s[:])
                m2 = mp.tile([TB, 1], F32, tag="m2")
                nc.vector.tensor_reduce(m2[:], msk2[:], axis=AX.X, op=ALU.max)
                e2 = mp.tile([TB, E], F32, tag="e2")
                nc.vector.tensor_scalar(e2[:], msk2[:], m2[:], None, op0=ALU.is_equal)
                emask = mp.tile([TB, E], F32, tag="emask")
                nc.vector.tensor_add(emask[:], e1[:], e2[:])
                den = mp.tile([TB, 1], F32, tag="den")
                nc.vector.tensor_add(den[:], m1[:], m2[:])
                dinv = mp.tile([TB, 1], F32, tag="dinv")
                nc.vector.reciprocal(dinv[:], den[:])
                wte = mp.tile([TB, E], F32, tag="wte")
                nc.vector.tensor_mul(wte[:], rw_sb[:], emask[:])
                nc.vector.tensor_scalar_mul(wte[:], wte[:], dinv[:])
                xpa = mp.tile([TB, E + 1024], BF16, tag="xpa")
                nc.vector.tensor_copy(xpa[:, 0:E], wte[:])
                nc.vector.tensor_copy(xpa[:, E:E + 1024], xm[:, 0:1024])
                nc.sync.dma_start(out=ag_xa_in[:, :], in_=xpa[:])
                nc.gpsimd.collective_compute(
                    "AllGather", ALU.bypass, replica_groups=rg,
                    ins=[ag_xa_in[:, :].opt()], outs=[ag_xa_out[:, :, :].opt()],
                )
                xpb = mp.tile([TB, 1024], BF16, tag="xpb")
                nc.vector.tensor_copy(xpb[:], xm[:, 1024:2048])
                nc.sync.dma_start(out=ag_xb_in[:, :], in_=xpb[:])
                nc.gpsimd.collective_compute(
                    "AllGather", ALU.bypass, replica_groups=rg,
                    ins=[ag_xb_in[:, :].opt()], outs=[ag_xb_out[:, :, :].opt()],
                )

            # =============== MoE expert compute (capacity gather) ===============
            with (
                tc.tile_pool(name="moe1", bufs=1) as m1p,
                tc.tile_pool(name="moew", bufs=6) as wp,
                tc.tile_pool(name="moed", bufs=3) as dp,
                tc.tile_pool(name="moet", bufs=2) as tp,
            ):
                # w_te columns first (small), so routing/dispatch overlaps
                # the big x_all DMA
                wcols = m1p.tile([128, NCORES, E], BF16, tag="wcols")
                for b in range(NCORES):
                    nc.sync.dma_start(out=wcols[:, b, :],
                                      in_=ag_xa_out[b, :, 0:E])
                # all tokens, token-major: [t, b, x] in two halves
                x_alla = m1p.tile([128, NCORES, 1024], BF16, tag="x_alla")
                for b in range(NCORES):
                    nc.sync.dma_start(out=x_alla[:, b, :],
                                      in_=ag_xa_out[b, :, E:E + 1024])
                x_allb = m1p.tile([128, NCORES, 1024], BF16, tag="x_allb")
                for b in range(NCORES):
                    nc.sync.dma_start(out=x_allb[:, b, :], in_=ag_xb_out[b, :, :])
                # --- routing indicator: is this core's expert in token's top-2 ---
                woc = m1p.tile([128, NCORES], F32, tag="woc")
                for b in range(NCORES):
                    t8 = tp.tile([128, E], F32, tag="t8")
                    nc.vector.tensor_mul(t8[:], wcols[:, b, :], esel_sb[:])
                    nc.vector.tensor_reduce(woc[:, b:b + 1], t8[:], axis=AX.X,
                                            op=ALU.add)
                ind_bf = m1p.tile([128, NCORES], BF16, tag="ind_bf")
                nc.vector.tensor_scalar(ind_bf[:], woc[:], 0.0, None,
                                        op0=ALU.is_gt)

                # --- capacity slot per routed token (exclusive running count) ---
                plc = psC.tile([128, 512], F32, tag="mid")
                nc.tensor.matmul(plc[:, 0:NCORES], ltri_sb[:], ind_bf[:],
                                 start=True, stop=True)
                posL = m1p.tile([128, NCORES], F32, tag="posL")
                nc.scalar.copy(posL[:], plc[:, 0:NCORES])
                ptc = psC.tile([128, 512], F32, tag="mid")
                nc.tensor.matmul(ptc[0:1, 0:NCORES], onesc_sb[:], ind_bf[:],
                                 start=True, stop=True)
                totf = m1p.tile([1, NCORES], F32, tag="totf")
                nc.scalar.copy(totf[:], ptc[0:1, 0:NCORES])
                # exclusive cumsum over the 8 chunk totals
                c1 = m1p.tile([1, NCORES], F32, tag="c1")
                nc.vector.memset(c1[:], 0.0)
                nc.vector.tensor_copy(c1[:, 1:8], totf[:, 0:7])
                c2 = m1p.tile([1, NCORES], F32, tag="c2")
                nc.vector.tensor_copy(c2[:, 0:1], c1[:, 0:1])
                nc.vector.tensor_add(c2[:, 1:8], c1[:, 1:8], c1[:, 0:7])
                c3 = m1p.tile([1, NCORES], F32, tag="c3")
                nc.vector.tensor_copy(c3[:, 0:2], c2[:, 0:2])
                nc.vector.tensor_add(c3[:, 2:8], c2[:, 2:8], c2[:, 0:6])
                c4 = m1p.tile([1, NCORES], F32, tag="c4")
                nc.vector.tensor_copy(c4[:, 0:4], c3[:, 0:4])
                nc.vector.tensor_add(c4[:, 4:8], c3[:, 4:8], c3[:, 0:4])
                poc = psC.tile([128, 512], F32, tag="mid")
                nc.tensor.matmul(poc[:, 0:NCORES], ones1f_sb[:], c4[:],
                                 start=True, stop=True)
                pos2 = m1p.tile([128, NCORES], F32, tag="pos2")
                nc.vector.tensor_add(pos2[:], posL[:], poc[:, 0:NCORES])
                nbig = m1p.tile([128, NCORES], F32, tag="nbig")
                nc.vector.tensor_scalar(nbig[:], ind_bf[:], -4096.0, 4096.0,
                                        op0=ALU.mult, op1=ALU.add)
                nc.vector.tensor_add(pos2[:], pos2[:], nbig[:])

                # --- one-hot dispatch P[t, c] and its transpose ---
                P = m1p.tile([128, NCORES, CAP], BF16, tag="P")
                for b in range(NCORES):
                    nc.vector.tensor_scalar(P[:, b, :], iotac_sb[:],
                                            pos2[:, b:b + 1], None,
                                            op0=ALU.is_equal)
                PT = m1p.tile([128, CB, S], BF16, tag="PT")
                for b in range(NCORES):
                    for j in range(CB):
                        ptb = psD.tile([128, 128], BF16, tag="midb")
                        nc.tensor.transpose(ptb[:], P[:, b, j * 128:(j + 1) * 128],
                                            identb[:])
                        nc.scalar.copy(PT[:, j, b * 128:(b + 1) * 128], ptb[:])

                # --- gather xg[h, c] = x^T P and this expert's weights ---
                xg = m1p.tile([128, HC, CAP], BF16, tag="xg")
                for kc in range(HC):
                    xa = x_alla if kc < 8 else x_allb
                    k0 = (kc % 8) * 128
                    gp = psC.tile([128, 512], F32, tag="mid")
                    for b in range(NCORES):
                        nc.tensor.matmul(gp[:, 0:CAP],
                                         xa[:, b, k0:k0 + 128],
                                         P[:, b, :], start=(b == 0),
                                         stop=(b == NCORES - 1))
                    nc.scalar.copy(xg[:, kc, :], gp[:, 0:CAP])
                wgp = psC.tile([128, 512], F32, tag="mid")
                for b in range(NCORES):
                    nc.tensor.matmul(wgp[0:E, 0:CAP], wcols[:, b, :],
                                     P[:, b, :], start=(b == 0),
                                     stop=(b == NCORES - 1))
                wg8 = m1p.tile([E, CAP], BF16, tag="wg8")
                nc.scalar.copy(wg8[:], wgp[0:E, 0:CAP])
                wbp = psC.tile([128, 512], F32, tag="mid")
                nc.tensor.matmul(wbp[:, 0:CAP], sel_sb[:], wg8[:],
                                 start=True, stop=True)
                wbc = m1p.tile([128, CAP], F32, tag="wbc")
                nc.scalar.copy(wbc[:], wbp[:, 0:CAP])

                # --- up / gate / silu over gathered tokens ---
                intert = m1p.tile([128, FT, CAP], BF16, tag="intert")
                for ft in range(FT):
                    ut = wp.tile([128, HC, 128], BF16, tag="w_up")
                    nc.scalar.dma_start(out=ut[:], in_=upw[ft, :, :, :])
                    gt = wp.tile([128, HC, 128], BF16, tag="w_up")
                    nc.scalar.dma_start(out=gt[:], in_=gatew[ft, :, :, :])
                    big = psA.tile([TB, S], F32, tag="big")
                    pu = big[:, 0:CAP]
                    pg = big[:, 512:512 + CAP]
                    for kc in range(HC):
                        nc.tensor.matmul(pu, ut[:, kc, :], xg[:, kc, :],
                                         start=(kc == 0), stop=(kc == HC - 1))
                    for kc in range(HC):
                        nc.tensor.matmul(pg, gt[:, kc, :], xg[:, kc, :],
                                         start=(kc == 0), stop=(kc == HC - 1))
                    sl = tp.tile([128, CAP], F32, tag="silu_t")
                    nc.scalar.activation(sl[:], pu, AF.Silu)
                    nc.vector.tensor_mul(sl[:], sl[:], pg)
                    nc.vector.tensor_mul(intert[:, ft, :], sl[:], wbc[:])

                # --- down proj (ygT[c, h]) + scatter + chunked ReduceScatter,
                #     pipelined over four 512-wide h quarters; each quarter's
                #     residual-add + output store rides behind its RS ---
                out_sb = m1p.tile([TB, H], F32, tag="out_sb")
                ygT = m1p.tile([128, CB, 512], BF16, tag="ygT")
                for hq in range(4):
                    for sub in range(512 // DH):
                        h0 = hq * 512 + sub * DH
                        dw = dp.tile([128, FT, DH], BF16, tag="w_dn")
                        nc.scalar.dma_start(
                            out=dw[:],
                            in_=downw[:, :, h0:h0 + DH].rearrange("f p h -> p f h"))
                        for j in range(CB):
                            big = psA.tile([TB, S], F32, tag="big")
                            pd = big[:, 0:DH]
                            for ft in range(FT):
                                nc.tensor.matmul(
                                    pd, intert[:, ft, j * 128:(j + 1) * 128],
                                    dw[:, ft, :], start=(ft == 0),
                                    stop=(ft == FT - 1))
                            nc.scalar.copy(ygT[:, j, sub * DH:sub * DH + DH], pd)
                    # scatter this quarter for every token chunk, then RS it
                    for b in range(NCORES):
                        ys = tp.tile([128, 512], BF16, tag="y_sb")
                        big = psA.tile([TB, S], F32, tag="big")
                        ps = big[:, 0:512]
                        for j in range(CB):
                            nc.tensor.matmul(ps, PT[:, j, b * 128:(b + 1) * 128],
                                             ygT[:, j, :],
                                             start=(j == 0), stop=(j == CB - 1))
                        nc.vector.tensor_copy(ys[:], ps)
                        nc.sync.dma_start(out=y_part[hq][b, :, :], in_=ys[:])
                    nc.gpsimd.collective_compute(
                        "ReduceScatter", ALU.add, replica_groups=rg,
                        ins=[y_part[hq][:, :, :].opt()],
                        outs=[y_rs[hq][:, :].opt()],
                    )
                    # final out = x2 + y for this quarter, overlapping the
                    # next quarter's compute
                    yq = tp.tile([TB, 512], BF16, tag="yq")
                    nc.sync.dma_start(out=yq[:], in_=y_rs[hq][:, :])
                    nc.vector.tensor_add(out_sb[:, hq * 512:(hq + 1) * 512],
                                         x2[:, hq * 512:(hq + 1) * 512], yq[:])
                    nc.sync.dma_start(out=out_ext[:, hq * 512:(hq + 1) * 512],
                                      in_=out_sb[:, hq * 512:(hq + 1) * 512])

    nc.finalize()
    return nc


_NC_CACHE = None


def kernel(**inputs) -> np.ndarray:
    global _NC_CACHE
    hidden = np.asarray(inputs["hidden_states"], np.float32).reshape(S, H)
    amask = np.asarray(inputs["attention_mask"]).reshape(-1)[:S].astype(bool)
    cos = np.asarray(inputs["cos"], np.float32).reshape(S, HD)
    sin = np.asarray(inputs["sin"], np.float32).reshape(S, HD)
    q_w = np.asarray(inputs["q_w"], np.float32)
    k_w = np.asarray(inputs["k_w"], np.float32)
    v_w = np.asarray(inputs["v_w"], np.float32)
    o_w = np.asarray(inputs["o_w"], np.float32)
    ln1 = np.asarray(inputs["ln1_w"], np.float32)
    ln2 = np.asarray(inputs["ln2_w"], np.float32)
    router_w = np.asarray(inputs["router_w"], np.float32)
    up_w = np.asarray(inputs["up_w"], np.float32)
    gate_w = np.asarray(inputs["gate_w"], np.float32)
    down_w = np.asarray(inputs["down_w"], np.float32)

    scale = HD ** -0.5
    BF = ml_dtypes.bfloat16
    ident = np.eye(128, dtype=np.float32)
    identb = np.eye(128, dtype=BF)
    ltri = (np.arange(128)[:, None] < np.arange(128)[None, :]).astype(BF)
    onesc = np.ones((128, 1), BF)
    ones1f = np.ones((1, 128), np.float32)
    iotac = np.broadcast_to(np.arange(CAP, dtype=np.float32), (128, CAP)).copy()
    def retile_w(w):
        d = w.shape[1]
        return np.ascontiguousarray(
            w.reshape(HC, 128, d // 512, 512).transpose(2, 1, 0, 3).astype(BF))

    qw_f = retile_w(ln1[:, None] * q_w)
    kw_f = retile_w(ln1[:, None] * k_w)
    vw_f = retile_w(ln1[:, None] * v_w)
    ow_f = retile_w(o_w)
    rw_f = np.ascontiguousarray(ln2[:, None] * router_w)

    tri = np.where(np.arange(TB)[None, :] <= np.arange(TB)[:, None], 0.0,
                   NEG).astype(np.float32)

    if _NC_CACHE is None:
        _NC_CACHE = build_nc()
    nc = _NC_CACHE

    in_maps = []
    for c in range(NCORES):
        t0 = c * TB
        cos_c = cos[t0:t0 + TB]
        sin_c = sin[t0:t0 + TB]
        bias_arr = np.zeros((NCORES, TB, TB), np.float32)
        for b in range(NCORES):
            if b == c:
                bias_arr[b] = tri
            elif b > c:
                bias_arr[b] = NEG
            bias_arr[b][:, ~amask[b * TB:(b + 1) * TB]] = NEG
        sel = np.zeros((E, 128), BF)
        sel[c, :] = 1.0
        esel = np.zeros((128, E), BF)
        esel[:, c] = 1.0
        upw_t = np.ascontiguousarray(
            (ln2[:, None] * up_w[c]).reshape(HC, 128, FT, 128)
            .transpose(2, 1, 0, 3).astype(BF))
        gatew_t = np.ascontiguousarray(
            (ln2[:, None] * gate_w[c]).reshape(HC, 128, FT, 128)
            .transpose(2, 1, 0, 3).astype(BF))
        downw_t = np.ascontiguousarray(down_w[c].reshape(FT, 128, H).astype(BF))
        in_maps.append({
            "h": np.ascontiguousarray(hidden[t0:t0 + TB]),
            "cos_q": np.ascontiguousarray(np.tile(cos_c, (1, NH)) * scale),
            "sin_q": np.ascontiguousarray(np.tile(sin_c, (1, NH)) * scale),
            "cos_k": np.ascontiguousarray(np.tile(cos_c, (1, KVH))),
            "sin_k": np.ascontiguousarray(np.tile(sin_c, (1, KVH))),
            "bias_all": bias_arr,
            "ident": ident,
            "identb": identb,
            "sel": sel,
            "esel": esel,
            "ltri": ltri,
            "onesc": onesc,
            "ones1f": ones1f,
            "iotac": iotac,
            "qw": qw_f, "kw": kw_f, "vw": vw_f, "ow": ow_f, "rw": rw_f,
            "upw": upw_t, "gatew": gatew_t, "downw": downw_t,
        })

    trace = os.environ.get("KERNEL_TRACE", "0") == "1"
    res = run_bass_kernel_spmd(nc, in_maps, core_ids=list(range(NCORES)), trace=trace)
    kernel.last_result = res
    out = np.concatenate([res.results[c]["out"] for c in range(NCORES)], axis=0)
    return out.reshape(B, S, H).astype(np.float32)

